# revision 24
# baseline (speedup 1.0000x reference)
"""Trainium2 Bass kernel for nn_Encoder_67980742361889 (GAT + 2-layer
transformer encoder, dual-branch DGI-style head), SPMD across 8 NeuronCores.

Sharding: nodes (rows) are split 375/core.  GAT edges are sharded by dst,
bin-packed so each 128-edge bin holds only whole dst-segments; duplicate
edges merge into multiplicities (identical logits => exp log-bias).
Weights are replicated; K/V (and the xl table / final embeddings) are
exchanged with AllGather collectives.  Matmuls run in bf16 (fp32 PSUM).

Self-contained: call kernel(**inputs) with the full-size inputs; returns the
full (embedding, decoded, logits, logits_a) tuple.
"""
import os
import sys

sys.path.insert(0, "/opt/trn_rl_repo")

from contextlib import ExitStack

import numpy as np
import ml_dtypes

import concourse.bass as bass
import concourse.tile as tile
from concourse import mybir
from concourse import bacc
from concourse.bass_utils import run_bass_kernel_spmd
from concourse.masks import make_identity

# ---------------------------------------------------------------- constants
N = 3000
F_IN = 3000
D = 512
H = 8
CH = 64
DFF = 2048
NCORES = 8
R = N // NCORES            # 375 rows / core
BIN = 128
SLOTS = 16
LRELU_GAT = 0.2
LRELU_ACT = 0.01
NEG_BIG = -1.0e30

FP = mybir.dt.float32
BF = mybir.dt.bfloat16
I32 = mybir.dt.int32
AXX = mybir.AxisListType.X
AF = mybir.ActivationFunctionType
ALU = mybir.AluOpType

KCH = [128] * 23 + [56]            # 3000 contraction chunks
MCH = [128, 128, 119]              # 375 row chunks
BF_NP = ml_dtypes.bfloat16

_CACHE = {}
LAST_RESULT = None


# ------------------------------------------------------------- host edge prep
def _prep_edges(edge_index):
    src = np.asarray(edge_index[0], dtype=np.int64)
    dst = np.asarray(edge_index[1], dtype=np.int64)
    loops = np.arange(N, dtype=np.int64)
    src = np.concatenate([src, loops])
    dst = np.concatenate([dst, loops])
    key = dst * N + src
    uniq, counts = np.unique(key, return_counts=True)
    u_dst = (uniq // N).astype(np.int64)
    u_src = (uniq % N).astype(np.int64)
    per_core = []
    for k in range(NCORES):
        lo = k * R
        m = (u_dst >= lo) & (u_dst < lo + R)
        s_src = u_src[m]
        s_dst = u_dst[m] - lo
        s_mult = counts[m].astype(np.float64)
        order = np.argsort(s_dst, kind="stable")
        s_src, s_dst, s_mult = s_src[order], s_dst[order], s_mult[order]
        segs = []
        start = 0
        for i in range(1, len(s_dst) + 1):
            if i == len(s_dst) or s_dst[i] != s_dst[start]:
                segs.append((int(s_dst[start]), start, i))
                start = i
        bins = []
        cur, cur_e = [], 0
        for seg in segs:
            ln = seg[2] - seg[1]
            assert ln <= BIN
            if cur and (cur_e + ln > BIN or len(cur) >= SLOTS):
                bins.append(cur)
                cur, cur_e = [], 0
            cur.append(seg)
            cur_e += ln
        if cur:
            bins.append(cur)
        per_core.append((bins, s_src, s_mult))
    nbins = max(len(b) for b, _, _ in per_core)

    out = []
    for k in range(NCORES):
        bins, s_src, s_mult = per_core[k]
        Q = np.zeros((nbins, BIN, SLOTS), np.float32)
        QT = np.zeros((nbins, SLOTS, BIN), np.float32)
        srcg = np.zeros((nbins, BIN, 1), np.int32)
        slotd = np.full((nbins, SLOTS, 1), R, np.int32)     # pad -> trash row R
        logm = np.full((nbins, BIN, 1), NEG_BIG, np.float32)
        for b, seg_list in enumerate(bins):
            e = 0
            for s, (dloc, i0, i1) in enumerate(seg_list):
                slotd[b, s, 0] = dloc
                for i in range(i0, i1):
                    Q[b, e, s] = 1.0
                    QT[b, s, e] = 1.0
                    srcg[b, e, 0] = s_src[i]
                    logm[b, e, 0] = np.log(s_mult[i])
                    e += 1
        nkc = (nbins * SLOTS + 127) // 128
        Ep = np.zeros((nbins, 3, 128, BIN), np.float32)
        Pt = np.zeros((nkc, 128, 384), np.float32)
        for b in range(nbins):
            for e in range(BIN):
                s = int(np.argmax(Q[b, e])) if Q[b, e].any() else -1
                if s >= 0:
                    dl = int(slotd[b, s, 0])
                    Ep[b, dl // 128, dl % 128, e] = 1.0
        for b in range(nbins):
            for s in range(SLOTS):
                dl = int(slotd[b, s, 0])
                if dl < R:
                    gs = b * SLOTS + s
                    Pt[gs // 128, gs % 128, dl] = 1.0
        out.append(dict(Q=Q.astype(BF_NP), srcg=srcg, logm=logm,
                        Ep=Ep.astype(BF_NP), Pt=Pt.astype(BF_NP)))
    return out, nbins


# ----------------------------------------------------------------- builder
def _build(nbins):
    nc = bacc.Bacc(target_bir_lowering=False)
    NKC = (nbins * SLOTS + 127) // 128

    def din(name, shape, dt=BF):
        return nc.declare_dram_parameter(name, list(shape), dt, isOutput=False)[:]

    def dout(name, shape, dt=FP):
        return nc.declare_dram_parameter(name, list(shape), dt, isOutput=True)[:]

    xT = {0: din("xTa", [F_IN, R]), 1: din("xTb", [F_IN, R])}
    adjT = din("adjT", [N, R])
    rsr = din("rsrecip", [R, 1], FP)
    Wl = din("Wl", [F_IN, D]); Wr = din("Wr", [F_IN, D]); Wp = din("Wproj", [F_IN, D])
    WqkvT = din("WqkvT", [2, D, 3 * D])
    bqkvT = din("bqkvT", [2, 3 * D, 1], FP)
    WoT = din("WoT", [2, D, D])
    Wff1 = din("Wff1", [2, D, DFF])
    bff1T = din("bff1T", [2, DFF, 1], FP)
    Wff2 = din("Wff2", [2, DFF, D])
    Wout = din("Wout", [D, F_IN])
    WdT = din("WdiscT", [D, D])
    rows = din("rows", [1, 17 * D + F_IN])        # bf16, packing per _pack_rows
    sca = din("sca", [1, 4], FP)                  # a, 1-a, bdisc, 0
    Qm = din("Qm", [nbins, BIN, SLOTS])
    Epm = din("Epm", [nbins, 3, 128, BIN])
    Ptm = din("Ptm", [NKC, 128, 384])
    srcA = din("srcA", [BIN, nbins], I32)
    srcB = din("srcB", [BIN, nbins], I32)
    logm = din("logm", [nbins, BIN, 1], FP)

    emb_o = dout("emb_o", [R, D])
    dec_o = dout("dec_o", [R, F_IN])
    lg_o = dout("lg_o", [R, 2])
    lga_o = dout("lga_o", [R, 2])

    RG = [list(range(NCORES))]

    def bcast(ap2d, parts):
        return bass.AP(tensor=ap2d.tensor, offset=ap2d.offset,
                       ap=[[0, parts]] + [list(x) for x in ap2d.ap[1:]])

    with tile.TileContext(nc) as tc, ExitStack() as ctx:
        dram = ctx.enter_context(tc.tile_pool(name="dram", bufs=1, space="DRAM"))
        const = ctx.enter_context(tc.tile_pool(name="const", bufs=1))
        persist = ctx.enter_context(tc.tile_pool(name="persist", bufs=1))

        # DRAM internals
        xl_bounce = dram.tile([2, R, D], BF)
        xl_ag = dram.tile([NCORES, 2, R, D], BF, addr_space="Shared")
        xl_flat = xl_ag.rearrange("a b c d -> (a b c) d")
        gat_stage = dram.tile([2, NKC * 128, D], BF)
        kv_bounce = {}
        kv_ag = {}
        for l in range(2):
            for br in range(2):
                kv_bounce[l, br] = dram.tile([2 * D * R], BF, name=f"kvb{l}{br}")
                kv_ag[l, br] = dram.tile([NCORES, 2 * D * R], BF,
                                         addr_space="Shared", name=f"kvg{l}{br}")
        emb_bounce = dram.tile([2, R, D], BF)
        emb_ag = dram.tile([NCORES, 2, R, D], BF, addr_space="Shared")

        # constants
        ident = const.tile([128, 128], BF)
        make_identity(nc, ident[:])
        row_specs = [
            ("bproj", D), ("att", D), ("gatb", D),
            ("bv0", D), ("bv1", D), ("bo0", D), ("bo1", D),
            ("bf2_0", D), ("bf2_1", D),
            ("l1g0", D), ("l1b0", D), ("l2g0", D), ("l2b0", D),
            ("l1g1", D), ("l1b1", D), ("l2g1", D), ("l2b1", D),
        ]
        off = 0
        row_tiles = {}
        for nm, w in row_specs:
            t = const.tile([128, w], BF, name=f"row_{nm}")
            nc.sync.dma_start(out=t[:], in_=bcast(rows[:, off:off + w], 128))
            row_tiles[nm] = t
            off += w
        bout_off = off
        sca_t = const.tile([128, 4], FP)
        nc.sync.dma_start(out=sca_t[:], in_=bcast(sca[:, :], 128))
        tail = NKC * 128 - nbins * SLOTS
        if tail > 0:
            zt = const.tile([128, D], BF)
            nc.vector.memset(zt[:], 0.0)
            for br in (0, 1):
                nc.sync.dma_start(
                    out=gat_stage[br, nbins * SLOTS:NKC * 128, :],
                    in_=zt[:tail, :])

        # persistent activations (bf16)
        x_res = {br: [persist.tile([128, D], BF, name=f"xres{br}{m}", bufs=1)
                      for m in range(3)] for br in (0, 1)}
        emb_bf = {br: [persist.tile([128, D], BF, name=f"emb{br}{m}", bufs=1)
                       for m in range(3)] for br in (0, 1)}
        comb_A = [persist.tile([128, D], BF, name=f"combA{m}", bufs=1)
                  for m in range(3)]
        g_ro = {br: [persist.tile([128, D], BF, name=f"gro{br}{m}", bufs=1)
                     for m in range(3)] for br in (0, 1)}
        xr_sb = {br: [persist.tile([128, D], BF, name=f"xrsb{br}{m}", bufs=1)
                      for m in range(3)] for br in (0, 1)}

        # ============================================================ P1
        with tc.tile_pool(name="p1x", bufs=1) as p1x, \
             tc.tile_pool(name="p1w", bufs=4) as p1w, \
             tc.tile_pool(name="p1o", bufs=4) as p1o, \
             tc.tile_pool(name="p1ps", bufs=1, space="PSUM") as p1ps:
            xt_tiles = {}
            for br in (0, 1):
                for ki, kw in enumerate(KCH):
                    t = p1x.tile([128, R], BF, name=f"p1x{br}_{ki}", bufs=1)
                    nc.sync.dma_start(out=t[:kw, :],
                                      in_=xT[br][ki * 128:ki * 128 + kw, :])
                    xt_tiles[br, ki] = t

            def proj_pass(W, consume):
                ps = {(br, m): p1ps.tile([128, D], FP, name=f"p1ps{br}{m}",
                                         tag=f"p1ps{br}{m}", bufs=1)
                      for br in (0, 1) for m in range(3)}
                for ki, kw in enumerate(KCH):
                    wt = p1w.tile([128, D], BF, tag="p1w", name="wt")
                    nc.sync.dma_start(out=wt[:kw, :], in_=W[ki * 128:ki * 128 + kw, :])
                    for br in (0, 1):
                        mo = 0
                        for m, mw in enumerate(MCH):
                            nc.tensor.matmul(
                                out=ps[br, m][:mw, :],
                                lhsT=xt_tiles[br, ki][:kw, mo:mo + mw],
                                rhs=wt[:kw, :],
                                start=(ki == 0), stop=(ki == len(KCH) - 1))
                            mo += mw
                for br in (0, 1):
                    for m, mw in enumerate(MCH):
                        consume(br, m, mw, ps[br, m])

            def use_xl(br, m, mw, ps):
                t = p1o.tile([128, D], BF, tag="p1o", name="t")
                nc.vector.tensor_copy(out=t[:mw, :], in_=ps[:mw, :])
                nc.sync.dma_start(out=xl_bounce[br, m * 128:m * 128 + mw, :],
                                  in_=t[:mw, :])
            proj_pass(Wl, use_xl)
            nc.gpsimd.collective_compute(
                "AllGather", ALU.bypass, replica_groups=RG,
                ins=[xl_bounce[:].opt()], outs=[xl_ag[:].opt()])

            def use_xp(br, m, mw, ps):
                nc.vector.tensor_tensor(out=x_res[br][m][:mw, :], in0=ps[:mw, :],
                                        in1=row_tiles["bproj"][:mw, :], op=ALU.add)
            proj_pass(Wp, use_xp)

            def use_xr(br, m, mw, ps):
                nc.vector.tensor_copy(out=xr_sb[br][m][:mw, :], in_=ps[:mw, :])
            proj_pass(Wr, use_xr)

        # ============================================================ P2+P3
        def layernorm(pool, big, x_in, mw, g_row, b_row, out_bf):
            st = pool.tile([128, 6], FP, tag="ln_st", name="st")
            nc.vector.bn_stats(out=st[:mw, :], in_=x_in[:mw, :])
            mv = pool.tile([128, 2], FP, tag="ln_mv", name="mv")
            nc.vector.bn_aggr(out=mv[:mw, :], in_=st[:mw, :])
            tv = pool.tile([128, 1], FP, tag="ln_tv", name="tv")
            nc.vector.tensor_scalar_add(out=tv[:mw, :], in0=mv[:mw, 1:2],
                                        scalar1=1e-5)
            sd = pool.tile([128, 1], FP, tag="ln_sd", name="sd")
            nc.scalar.sqrt(out=sd[:mw, :], in_=tv[:mw, :])
            rstd = pool.tile([128, 1], FP, tag="ln_rs", name="rstd")
            nc.vector.reciprocal(out=rstd[:mw, :], in_=sd[:mw, :])
            xc = big.tile([128, D], FP, tag="scr", name="xc")
            nc.vector.scalar_tensor_tensor(
                out=xc[:mw, :], in0=x_in[:mw, :], scalar=mv[:mw, 0:1],
                in1=g_row[:mw, :], op0=ALU.subtract, op1=ALU.mult)
            nc.vector.scalar_tensor_tensor(
                out=out_bf[:mw, :], in0=xc[:mw, :], scalar=rstd[:mw, :],
                in1=b_row[:mw, :], op0=ALU.mult, op1=ALU.add)

        with tc.tile_pool(name="g_sb", bufs=2) as gsb, \
             tc.tile_pool(name="g_eb", bufs=4) as geb, \
             tc.tile_pool(name="g_idx", bufs=3) as gidx, \
             tc.tile_pool(name="t_one", bufs=1) as tone, \
             tc.tile_pool(name="t_w", bufs=1) as twr, \
             tc.tile_pool(name="t_ws", bufs=4) as tws, \
             tc.tile_pool(name="t_kt", bufs=1) as tkt, \
             tc.tile_pool(name="t_vs", bufs=1) as tvs, \
             tc.tile_pool(name="t_sc", bufs=4) as tsc, \
             tc.tile_pool(name="t_sm", bufs=2) as tsm, \
             tc.tile_pool(name="t_ps", bufs=1, space="PSUM") as tps:

            # ---------------- GAT bins (scheduler interleaves with P3)
            sidx_all = {}
            for br in (0, 1):
                t = gidx.tile([BIN, nbins], I32, name=f"sidxall{br}", bufs=1)
                nc.sync.dma_start(out=t[:], in_=(srcA if br == 0 else srcB))
                sidx_all[br] = t
            for br in (0, 1):
                for b in range(nbins):
                    lm = gidx.tile([BIN, 1], FP, tag="lm", name="lm")
                    nc.sync.dma_start(out=lm[:], in_=logm[b])
                    qb = gsb.tile([BIN, SLOTS], BF, tag="qb", name="qb")
                    nc.sync.dma_start(out=qb[:], in_=Qm[b])
                    ept = gsb.tile([128, 3, BIN], BF, tag="ept", name="ept")
                    nc.sync.dma_start(out=ept[:],
                                      in_=Epm[b].rearrange("kc p e -> p kc e"))
                    xlg = gsb.tile([BIN, D], BF, tag="xlg", name="xlg", bufs=4)
                    nc.gpsimd.indirect_dma_start(
                        out=xlg[:], out_offset=None,
                        in_=xl_flat,
                        in_offset=bass.IndirectOffsetOnAxis(
                            ap=sidx_all[br][:, b:b + 1], axis=0))

                    ps_z = tps.tile([BIN, D], FP, tag="gp", name="ps_z", bufs=2)
                    for kc, kw in enumerate(MCH):
                        nc.tensor.matmul(out=ps_z[:], lhsT=ept[:kw, kc, :],
                                         rhs=xr_sb[br][kc][:kw, :],
                                         start=(kc == 0), stop=(kc == 2))
                    z = geb.tile([BIN, D], BF, tag="ebuf", name="z")
                    nc.vector.tensor_tensor(out=z[:], in0=ps_z[:], in1=xlg[:],
                                            op=ALU.add)
                    lr = geb.tile([BIN, D], BF, tag="ebuf", name="lr")
                    nc.vector.scalar_tensor_tensor(
                        out=lr[:], in0=z[:], scalar=LRELU_GAT, in1=z[:],
                        op0=ALU.mult, op1=ALU.max)
                    tm = geb.tile([BIN, D], BF, tag="ebuf", name="tm")
                    nc.vector.tensor_tensor(out=tm[:], in0=lr[:],
                                            in1=row_tiles["att"][:], op=ALU.mult)
                    lgt = gidx.tile([BIN, H], FP, tag="lgt", name="lgt")
                    nc.vector.reduce_sum(
                        out=lgt[:, :, None],
                        in_=tm[:].rearrange("p (h c) -> p h c", h=H), axis=AXX)
                    w = gidx.tile([BIN, H], BF, tag="w", name="w")
                    nc.scalar.activation(out=w[:], in_=lgt[:], func=AF.Exp,
                                         bias=lm[:, :1], scale=1.0)
                    v = geb.tile([BIN, D], BF, tag="ebuf", name="v")
                    nc.vector.tensor_tensor(
                        out=v[:].rearrange("p (h c) -> p h c", h=H),
                        in0=xlg[:].rearrange("p (h c) -> p h c", h=H),
                        in1=w[:].to_broadcast([BIN, H, CH]), op=ALU.mult)
                    ps_u = tps.tile([SLOTS, D], FP, tag="gp", name="ps_u", bufs=2)
                    nc.tensor.matmul(out=ps_u[:], lhsT=qb[:], rhs=v[:],
                                     start=True, stop=True)
                    ps_s = tps.tile([SLOTS, H], FP, tag="gp", name="ps_s", bufs=2)
                    nc.tensor.matmul(out=ps_s[:], lhsT=qb[:], rhs=w[:],
                                     start=True, stop=True)
                    sse = gidx.tile([SLOTS, H], FP, tag="sse", name="sse")
                    nc.vector.tensor_scalar_add(out=sse[:], in0=ps_s[:],
                                                scalar1=1e-30)
                    rq = gidx.tile([SLOTS, H], FP, tag="rq", name="rq")
                    nc.vector.reciprocal(out=rq[:], in_=sse[:])
                    outr = gsb.tile([SLOTS, D], BF, tag="outr", name="outr")
                    nc.vector.tensor_tensor(
                        out=outr[:].rearrange("p (h c) -> p h c", h=H),
                        in0=ps_u[:].rearrange("p (h c) -> p h c", h=H),
                        in1=rq[:].to_broadcast([SLOTS, H, CH]), op=ALU.mult)
                    nc.sync.dma_start(
                        out=gat_stage[br, b * SLOTS:(b + 1) * SLOTS, :],
                        in_=outr[:])

            # ---------------- transformer
            tT = [tone.tile([128, R], BF, name=f"tT{f}", bufs=1) for f in range(4)]
            qT_t = {br: [tone.tile([128, R], BF, name=f"qT{br}_{f}", bufs=1)
                         for f in range(4)] for br in (0, 1)}
            oT_t = [tone.tile([128, R], BF, name=f"oT_t{f}", bufs=1)
                    for f in range(4)]
            kT_t = [tone.tile([128, R], BF, name=f"kT_t{f}", bufs=1)
                    for f in range(4)]
            rT = [tone.tile([128, R], BF, name=f"rT{f}", bufs=1) for f in range(8)]

            def transpose_rows(psp, src_tiles, dst_tiles, pbufs=3):
                mo = 0
                for m, mw in enumerate(MCH):
                    for f in range(4):
                        pt = psp.tile([128, 128], BF, tag="pb", name="pt",
                                      bufs=pbufs)
                        nc.tensor.transpose(
                            out=pt[:, :mw],
                            in_=src_tiles[m][:mw, f * 128:(f + 1) * 128],
                            identity=ident[:mw, :mw])
                        nc.vector.tensor_copy(out=dst_tiles[f][:, mo:mo + mw],
                                              in_=pt[:, :mw])
                    mo += mw

            for l in range(2):
                wq4 = [twr.tile([128, 3 * D], BF, name=f"wq{dk}", tag=f"wq{dk}",
                                bufs=1) for dk in range(4)]
                for dk in range(4):
                    nc.sync.dma_start(out=wq4[dk][:],
                                      in_=WqkvT[l, dk * 128:(dk + 1) * 128, :])
                wo4 = [twr.tile([128, D], BF, name=f"wo{dk}", tag=f"wo{dk}",
                                bufs=1) for dk in range(4)]
                for dk in range(4):
                    nc.sync.dma_start(out=wo4[dk][:],
                                      in_=WoT[l, dk * 128:(dk + 1) * 128, :])
                wf1 = [twr.tile([128, DFF], BF, name=f"wf1_{dk}", tag=f"wf1_{dk}",
                                bufs=1) for dk in range(4)]
                for dk in range(4):
                    nc.sync.dma_start(out=wf1[dk][:],
                                      in_=Wff1[l, dk * 128:(dk + 1) * 128, :])
                wf2 = [twr.tile([128, D], BF, name=f"wf2_{c}", tag=f"wf2_{c}",
                                bufs=1) for c in range(16)]
                for c in range(16):
                    nc.sync.dma_start(out=wf2[c][:],
                                      in_=Wff2[l, c * 128:(c + 1) * 128, :])
                bq_sl = {}
                for part in range(3):
                    for f in range(4):
                        t = tsm.tile([128, 1], FP, tag=f"bq{part}{f}",
                                     name="t", bufs=1)
                        nc.sync.dma_start(
                            out=t[:],
                            in_=bqkvT[l, part * D + f * 128:
                                      part * D + (f + 1) * 128, :])
                        bq_sl[part, f] = t
                bf1_sl = {}
                for c in range(16):
                    t = tsm.tile([128, 1], FP, tag=f"bf1{c}", name="t", bufs=1)
                    nc.sync.dma_start(out=t[:],
                                      in_=bff1T[l, c * 128:(c + 1) * 128, :])
                    bf1_sl[c] = t

                # QKV + AG for both branches first (hides AG latency)
                for br in (0, 1):
                    transpose_rows(tps, x_res[br], tT)
                    for part, dest in ((0, qT_t[br]), (1, kT_t)):
                        for f in range(4):
                            ps = tps.tile([128, R], FP, tag="pb", name="ps",
                                          bufs=3)
                            for dk in range(4):
                                nc.tensor.matmul(
                                    out=ps[:],
                                    lhsT=wq4[dk][:, part * D + f * 128:
                                                 part * D + (f + 1) * 128],
                                    rhs=tT[dk][:],
                                    start=(dk == 0), stop=(dk == 3))
                            nc.vector.tensor_scalar_add(
                                out=dest[f][:], in0=ps[:],
                                scalar1=bq_sl[part, f][:, :1])
                    kvb = kv_bounce[l, br]
                    ktv = kvb.rearrange("(f t) -> f t", f=2 * D)
                    for f in range(4):
                        nc.sync.dma_start(out=ktv[f * 128:(f + 1) * 128, :],
                                          in_=kT_t[f][:])
                    mo = 0
                    for m, mw in enumerate(MCH):
                        ps = tps.tile([128, D], FP, tag="pb", name="ps", bufs=3)
                        for dk in range(4):
                            nc.tensor.matmul(
                                out=ps[:mw, :], lhsT=tT[dk][:, mo:mo + mw],
                                rhs=wq4[dk][:, 2 * D:3 * D],
                                start=(dk == 0), stop=(dk == 3))
                        vt = tsc.tile([128, D], BF, tag="scb", name="vt")
                        nc.vector.tensor_tensor(
                            out=vt[:mw, :], in0=ps[:mw, :],
                            in1=row_tiles[f"bv{l}"][:mw, :], op=ALU.add)
                        vv = kvb[D * R:].rearrange("(t f) -> t f", t=R)
                        nc.sync.dma_start(out=vv[mo:mo + mw, :], in_=vt[:mw, :])
                        mo += mw
                    nc.gpsimd.collective_compute(
                        "AllGather", ALU.bypass, replica_groups=RG,
                        ins=[kvb[:].opt()], outs=[kv_ag[l, br][:].opt()])

                # attention + oproj + LN1 per branch
                for br in (0, 1):
                    kvg = kv_ag[l, br]
                    kt4 = []
                    for blk in range(NCORES):
                        t = tkt.tile([128, 4, R], BF, name=f"kt4_{blk}",
                                     tag=f"kt4_{blk}", bufs=1)
                        nc.sync.dma_start(
                            out=t[:],
                            in_=kvg[blk, :D * R].rearrange(
                                "(c p t) -> p c t", c=4, p=128))
                        kt4.append(t)
                    vsb = []
                    for tb in range(24):
                        blk, sub = tb // 3, tb % 3
                        toff, tsubw = sub * 128, MCH[sub]
                        t = tvs.tile([128, H, CH + 1], BF, name=f"vsb{tb}",
                                     tag=f"vsb{tb}", bufs=1)
                        nc.vector.memset(t[:, :, CH:CH + 1], 1.0)
                        vv = kvg[blk, D * R:].rearrange("(t f) -> t f", t=R)
                        nc.sync.dma_start(
                            out=t[:tsubw, :, :CH],
                            in_=vv[toff:toff + tsubw, :].rearrange(
                                "t (h c) -> t h c", h=H))
                        vsb.append(t)
                    for h in range(H):
                        ps_o = tps.tile([65, R], FP, tag="po", name="ps_o",
                                        bufs=1)
                        for tb in range(24):
                            blk, sub = tb // 3, tb % 3
                            toff, tsubw = sub * 128, MCH[sub]
                            ps_s = tps.tile([128, R], FP, tag="psx",
                                            name="ps_s", bufs=2)
                            nc.tensor.matmul(
                                out=ps_s[:tsubw, :],
                                lhsT=kt4[blk][(h % 2) * 64:(h % 2) * 64 + 64,
                                              h // 2, toff:toff + tsubw],
                                rhs=qT_t[br][h // 2][(h % 2) * 64:
                                                     (h % 2) * 64 + 64, :],
                                start=True, stop=True)
                            eT = tsc.tile([128, R], BF, tag="eT", name="eT",
                                          bufs=2)
                            nc.scalar.activation(out=eT[:tsubw, :],
                                                 in_=ps_s[:tsubw, :],
                                                 func=AF.Exp, scale=0.125)
                            nc.tensor.matmul(
                                out=ps_o[:],
                                lhsT=vsb[tb][:tsubw, h, :],
                                rhs=eT[:tsubw, :],
                                start=(tb == 0), stop=(tb == 23))
                        rd = tsm.tile([1, R], BF, tag="rd", name="rd", bufs=2)
                        with nc.allow_low_precision(reason="softmax recip bf16"):
                            nc.vector.reciprocal(out=rd[:], in_=ps_o[64:65, :])
                        rb = tsm.tile([64, R], BF, tag="rb", name="rb", bufs=2)
                        nc.gpsimd.partition_broadcast(rb[:], rd[:])
                        nc.vector.tensor_tensor(
                            out=oT_t[h // 2][(h % 2) * 64:(h % 2) * 64 + 64, :],
                            in0=ps_o[:64, :], in1=rb[:], op=ALU.mult)
                    mo = 0
                    for m, mw in enumerate(MCH):
                        ps = tps.tile([128, D], FP, tag="pb", name="ps", bufs=3)
                        for dk in range(4):
                            nc.tensor.matmul(out=ps[:mw, :],
                                             lhsT=oT_t[dk][:, mo:mo + mw],
                                             rhs=wo4[dk][:],
                                             start=(dk == 0), stop=(dk == 3))
                        e2 = tsc.tile([128, D], FP, tag="scr", name="e2")
                        nc.vector.tensor_tensor(out=e2[:mw, :], in0=ps[:mw, :],
                                                in1=row_tiles[f"bo{l}"][:mw, :],
                                                op=ALU.add)
                        e3 = tsc.tile([128, D], FP, tag="scr", name="e3")
                        nc.vector.tensor_tensor(out=e3[:mw, :], in0=e2[:mw, :],
                                                in1=x_res[br][m][:mw, :],
                                                op=ALU.add)
                        layernorm(tsm, tsc, e3, mw, row_tiles[f"l1g{l}"],
                                  row_tiles[f"l1b{l}"], x_res[br][m])
                        mo += mw
                # FFN per branch (rT halves to bound SBUF)
                for br in (0, 1):
                    transpose_rows(tps, x_res[br], tT)
                    ffa = [tsc.tile([128, D], BF, tag=f"ffa{m}", name="ffa",
                                    bufs=1) for m in range(3)]
                    for half in range(2):
                        for ci in range(8):
                            c = half * 8 + ci
                            ps = tps.tile([128, R], FP, tag="pb", name="ps",
                                          bufs=3)
                            for dk in range(4):
                                nc.tensor.matmul(
                                    out=ps[:],
                                    lhsT=wf1[dk][:, c * 128:(c + 1) * 128],
                                    rhs=tT[dk][:],
                                    start=(dk == 0), stop=(dk == 3))
                            nc.scalar.activation(out=rT[ci][:], in_=ps[:],
                                                 func=AF.Relu,
                                                 bias=bf1_sl[c][:, :1], scale=1.0)
                        mo = 0
                        for m, mw in enumerate(MCH):
                            ps = tps.tile([128, D], FP, tag="pb", name="ps",
                                          bufs=3)
                            for ci in range(8):
                                nc.tensor.matmul(
                                    out=ps[:mw, :],
                                    lhsT=rT[ci][:, mo:mo + mw],
                                    rhs=wf2[half * 8 + ci][:],
                                    start=(ci == 0), stop=(ci == 7))
                            if half == 0:
                                nc.vector.tensor_copy(out=ffa[m][:mw, :],
                                                      in_=ps[:mw, :])
                            else:
                                e2 = tsc.tile([128, D], FP, tag="scr", name="e2")
                                nc.vector.tensor_tensor(
                                    out=e2[:mw, :], in0=ps[:mw, :],
                                    in1=ffa[m][:mw, :], op=ALU.add)
                                e2b = tsc.tile([128, D], FP, tag="scr",
                                               name="e2b")
                                nc.vector.tensor_tensor(
                                    out=e2b[:mw, :], in0=e2[:mw, :],
                                    in1=row_tiles[f"bf2_{l}"][:mw, :], op=ALU.add)
                                e3 = tsc.tile([128, D], FP, tag="scr", name="e3")
                                nc.vector.tensor_tensor(
                                    out=e3[:mw, :], in0=e2b[:mw, :],
                                    in1=x_res[br][m][:mw, :], op=ALU.add)
                                layernorm(tsm, tsc, e3, mw,
                                          row_tiles[f"l2g{l}"],
                                          row_tiles[f"l2b{l}"], x_res[br][m])
                            mo += mw

            # ==================================================== P4 comb
            for br in (0, 1):
                mo = 0
                for m, mw in enumerate(MCH):
                    pg = tps.tile([128, D], FP, tag="pb", name="pg", bufs=3)
                    for kc in range(NKC):
                        gs = tsc.tile([128, D], BF, tag="scb", name="gs")
                        nc.sync.dma_start(
                            out=gs[:],
                            in_=gat_stage[br, kc * 128:(kc + 1) * 128, :])
                        ptk = tws.tile([128, 384], BF, tag="w", name="ptk")
                        nc.sync.dma_start(out=ptk[:, :384], in_=Ptm[kc])
                        nc.tensor.matmul(out=pg[:mw, :],
                                         lhsT=ptk[:, mo:mo + mw],
                                         rhs=gs[:],
                                         start=(kc == 0), stop=(kc == NKC - 1))
                    g1 = tsc.tile([128, D], FP, tag="scr", name="g1")
                    nc.vector.tensor_tensor(out=g1[:mw, :], in0=pg[:mw, :],
                                            in1=row_tiles["gatb"][:mw, :],
                                            op=ALU.add)
                    xs = tsc.tile([128, D], FP, tag="scr", name="xs")
                    nc.vector.tensor_scalar_mul(out=xs[:mw, :],
                                                in0=x_res[br][m][:mw, :],
                                                scalar1=sca_t[:mw, 1:2])
                    cmb = tsc.tile([128, D], FP, tag="scr", name="cmb")
                    nc.vector.scalar_tensor_tensor(
                        out=cmb[:mw, :], in0=g1[:mw, :],
                        scalar=sca_t[:mw, 0:1],
                        in1=xs[:mw, :], op0=ALU.mult, op1=ALU.add)
                    embf = tsc.tile([128, D], FP, tag="scr", name="embf")
                    nc.vector.scalar_tensor_tensor(
                        out=embf[:mw, :], in0=cmb[:mw, :], scalar=LRELU_ACT,
                        in1=cmb[:mw, :], op0=ALU.mult, op1=ALU.max)
                    nc.vector.tensor_copy(out=emb_bf[br][m][:mw, :],
                                          in_=embf[:mw, :])
                    if br == 0:
                        nc.vector.tensor_copy(out=comb_A[m][:mw, :],
                                              in_=cmb[:mw, :])
                        nc.sync.dma_start(out=emb_o[mo:mo + mw, :],
                                          in_=embf[:mw, :])
                    nc.sync.dma_start(out=emb_bounce[br, mo:mo + mw, :],
                                      in_=emb_bf[br][m][:mw, :])
                    mo += mw
            nc.gpsimd.collective_compute(
                "AllGather", ALU.bypass, replica_groups=RG,
                ins=[emb_bounce[:].opt()], outs=[emb_ag[:].opt()])

            # ==================================================== P5 dec
            transpose_rows(tps, comb_A, tT)
            for nchi in range(6):
                wo_t = []
                for f in range(4):
                    t = tws.tile([128, 500], BF, tag="w", name="wt")
                    nc.sync.dma_start(
                        out=t[:],
                        in_=Wout[f * 128:(f + 1) * 128,
                                 nchi * 500:(nchi + 1) * 500])
                    wo_t.append(t)
                bo_sl = tsc.tile([128, 500], BF, tag="scb", name="bo_sl")
                nc.sync.dma_start(
                    out=bo_sl[:],
                    in_=bcast(rows[:, bout_off + nchi * 500:
                                   bout_off + (nchi + 1) * 500], 128))
                mo = 0
                for m, mw in enumerate(MCH):
                    ps = tps.tile([128, 500], FP, tag="pb", name="ps", bufs=3)
                    for f in range(4):
                        nc.tensor.matmul(out=ps[:mw, :],
                                         lhsT=tT[f][:, mo:mo + mw],
                                         rhs=wo_t[f][:, :500],
                                         start=(f == 0), stop=(f == 3))
                    dv = tsc.tile([128, 500], FP, tag="scr", name="dv")
                    nc.vector.tensor_tensor(
                        out=dv[:mw, :], in0=ps[:mw, :],
                        in1=bo_sl[:mw, :], op=ALU.add)
                    nc.sync.dma_start(
                        out=dec_o[mo:mo + mw, nchi * 500:(nchi + 1) * 500],
                        in_=dv[:mw, :])
                    mo += mw

            # ==================================================== P6 read
            rsr_sl = []
            for m, mw in enumerate(MCH):
                t = tsm.tile([128, 1], FP, tag=f"rsr{m}", name="t", bufs=1)
                nc.sync.dma_start(out=t[:mw, :],
                                  in_=rsr[m * 128:m * 128 + mw, :])
                rsr_sl.append(t)
            _ro_tags = {(0, 0): ("pb", 3), (0, 1): ("pb", 3), (0, 2): ("pb", 3),
                        (1, 0): ("po", 1), (1, 1): ("gp", 2), (1, 2): ("psx", 2)}
            psro = {(br, m): tps.tile([128, D], FP, name=f"ro{br}{m}",
                                      tag=_ro_tags[br, m][0],
                                      bufs=_ro_tags[br, m][1])
                    for br in (0, 1) for m in range(3)}
            for tb in range(24):
                blk, sub = tb // 3, tb % 3
                toff, tsubw = sub * 128, MCH[sub]
                at = tsc.tile([128, R], BF, tag="scb", name="at")
                nc.sync.dma_start(
                    out=at[:tsubw, :],
                    in_=adjT[blk * R + toff:blk * R + toff + tsubw, :])
                for br in (0, 1):
                    et = tsc.tile([128, D], BF, tag="scb2", name="et")
                    nc.sync.dma_start(out=et[:tsubw, :],
                                      in_=emb_ag[blk, br, toff:toff + tsubw, :])
                    mo = 0
                    for m, mw in enumerate(MCH):
                        nc.tensor.matmul(out=psro[br, m][:mw, :],
                                         lhsT=at[:tsubw, mo:mo + mw],
                                         rhs=et[:tsubw, :],
                                         start=(tb == 0), stop=(tb == 23))
                        mo += mw
            for br in (0, 1):
                for m, mw in enumerate(MCH):
                    cta = tsc.tile([128, D], FP, tag="scr", name="cta")
                    nc.vector.tensor_scalar_mul(out=cta[:mw, :],
                                                in0=psro[br, m][:mw, :],
                                                scalar1=rsr_sl[m][:mw, :1])
                    sq = tsc.tile([128, D], FP, tag="scr", name="sq")
                    nc.vector.tensor_tensor(out=sq[:mw, :], in0=cta[:mw, :],
                                            in1=cta[:mw, :], op=ALU.mult)
                    ss = tsm.tile([128, 1], FP, tag="ss", name="ss")
                    nc.vector.reduce_sum(out=ss[:mw, :], in_=sq[:mw, :],
                                         axis=AXX)
                    sr = tsm.tile([128, 1], FP, tag="sr", name="sr")
                    nc.scalar.sqrt(out=sr[:mw, :], in_=ss[:mw, :])
                    smx = tsm.tile([128, 1], FP, tag="smx", name="smx")
                    nc.vector.tensor_scalar_max(out=smx[:mw, :],
                                                in0=sr[:mw, :], scalar1=1e-12)
                    rn = tsm.tile([128, 1], FP, tag="rn", name="rn")
                    nc.vector.reciprocal(out=rn[:mw, :], in_=smx[:mw, :])
                    cn = tsc.tile([128, D], FP, tag="scr", name="cn")
                    nc.vector.tensor_scalar_mul(out=cn[:mw, :],
                                                in0=cta[:mw, :],
                                                scalar1=rn[:mw, :1])
                    nc.scalar.activation(out=g_ro[br][m][:mw, :],
                                         in_=cn[:mw, :], func=AF.Sigmoid)

            # ==================================================== P7 disc
            wd = []
            for e in range(4):
                t = tws.tile([128, D], BF, tag="w", name="wt")
                nc.sync.dma_start(out=t[:], in_=WdT[e * 128:(e + 1) * 128, :])
                wd.append(t)
            for br, out_t in ((0, lg_o), (1, lga_o)):
                transpose_rows(tps, g_ro[br], tT, pbufs=3)
                mo = 0
                for m, mw in enumerate(MCH):
                    ps = tps.tile([128, D], FP, tag="pb", name="ps", bufs=3)
                    for e in range(4):
                        nc.tensor.matmul(out=ps[:mw, :],
                                         lhsT=tT[e][:, mo:mo + mw],
                                         rhs=wd[e][:, :D],
                                         start=(e == 0), stop=(e == 3))
                    t1 = tsc.tile([128, D], FP, tag="scr", name="t1")
                    nc.vector.tensor_copy(out=t1[:mw, :], in_=ps[:mw, :])
                    lgt = tsm.tile([128, 2], FP, tag="lgt", name="lgt")
                    for col, ebr in ((0, br), (1, 1 - br)):
                        pr = tsc.tile([128, D], FP, tag="scr", name="pr")
                        nc.vector.tensor_tensor(out=pr[:mw, :],
                                                in0=emb_bf[ebr][m][:mw, :],
                                                in1=t1[:mw, :], op=ALU.mult)
                        s1 = tsm.tile([128, 1], FP, tag="s1", name="s1")
                        nc.vector.reduce_sum(out=s1[:mw, :], in_=pr[:mw, :],
                                             axis=AXX)
                        nc.vector.tensor_scalar_add(
                            out=lgt[:mw, col:col + 1], in0=s1[:mw, :],
                            scalar1=sca_t[:mw, 2:3])
                    nc.sync.dma_start(out=out_t[mo:mo + mw, :],
                                      in_=lgt[:mw, :])
                    mo += mw
    nc.compile()
    return nc


# ------------------------------------------------------------------ host API
def _pack_rows(inputs):
    f32 = np.float32
    z = []
    z.append(np.asarray(inputs["bproj"], f32).reshape(-1))
    z.append(np.asarray(inputs["att"], f32).reshape(-1))
    z.append(np.asarray(inputs["gat_b"], f32).reshape(-1))
    bqkv = np.asarray(inputs["bqkv"], f32)
    z.append(bqkv[0, 2 * D:3 * D]); z.append(bqkv[1, 2 * D:3 * D])
    bo = np.asarray(inputs["bo"], f32)
    z.append(bo[0]); z.append(bo[1])
    bff2 = np.asarray(inputs["bff2"], f32)
    z.append(bff2[0]); z.append(bff2[1])
    for l in range(2):
        z.append(np.asarray(inputs["ln1_g"], f32)[l])
        z.append(np.asarray(inputs["ln1_b"], f32)[l])
        z.append(np.asarray(inputs["ln2_g"], f32)[l])
        z.append(np.asarray(inputs["ln2_b"], f32)[l])
    z.append(np.asarray(inputs["bout"], f32).reshape(-1))
    return np.concatenate(z)[None, :]


def _make_in_maps(inputs):
    f32 = np.float32
    prep, nbins = _prep_edges(inputs["edge_index"])

    feat = np.asarray(inputs["feat"], f32)
    feat_a = np.asarray(inputs["feat_a"], f32)
    adj = np.asarray(inputs["adj_new"], f32)
    rows = _pack_rows(inputs).astype(BF_NP)
    a = 1.0 / (1.0 + np.exp(-float(np.asarray(inputs["alpha_param"]).reshape(-1)[0])))
    sca = np.array([[a, 1.0 - a, float(np.asarray(inputs["bdisc"])), 0.0]], f32)

    bf = BF_NP
    Wl = np.asarray(inputs["Wl"], f32).astype(bf)
    Wr = np.asarray(inputs["Wr"], f32).astype(bf)
    Wp = np.asarray(inputs["Wproj"], f32).astype(bf)
    Wqkv = np.asarray(inputs["Wqkv"], f32)
    WqkvT = np.ascontiguousarray(np.transpose(Wqkv, (0, 2, 1))).astype(bf)
    bqkvT = np.ascontiguousarray(np.asarray(inputs["bqkv"], f32)[:, :, None])
    Wo = np.asarray(inputs["Wo"], f32)
    WoT = np.ascontiguousarray(np.transpose(Wo, (0, 2, 1))).astype(bf)
    Wff1 = np.asarray(inputs["Wff1"], f32).astype(bf)
    bff1T = np.ascontiguousarray(np.asarray(inputs["bff1"], f32)[:, :, None])
    Wff2 = np.asarray(inputs["Wff2"], f32).astype(bf)
    Wout = np.asarray(inputs["Wout"], f32).astype(bf)
    WdiscT = np.ascontiguousarray(np.asarray(inputs["Wdisc"], f32).T).astype(bf)

    in_maps = []
    for k in range(NCORES):
        lo = k * R
        p = prep[k]
        sg = p["srcg"][:, :, 0].astype(np.int64)
        blk, subr = sg // R, sg % R
        srcA = np.ascontiguousarray((blk * (2 * R) + subr).astype(np.int32).T)
        srcB = np.ascontiguousarray((blk * (2 * R) + R + subr).astype(np.int32).T)
        rs = adj[lo:lo + R].sum(1, keepdims=True)
        in_maps.append(dict(
            xTa=np.ascontiguousarray(feat[lo:lo + R].T).astype(bf),
            xTb=np.ascontiguousarray(feat_a[lo:lo + R].T).astype(bf),
            adjT=np.ascontiguousarray(adj[lo:lo + R].T).astype(bf),
            rsrecip=(1.0 / rs).astype(f32),
            Wl=Wl, Wr=Wr, Wproj=Wp, WqkvT=WqkvT, bqkvT=bqkvT, WoT=WoT,
            Wff1=Wff1, bff1T=bff1T, Wff2=Wff2, Wout=Wout, WdiscT=WdiscT,
            rows=rows, sca=sca,
            Qm=p["Q"], Epm=p["Ep"], Ptm=p["Pt"], srcA=srcA, srcB=srcB,
            logm=p["logm"],
        ))
    return in_maps, nbins


def kernel(**inputs):
    in_maps, nbins = _make_in_maps(inputs)
    if nbins not in _CACHE:
        _CACHE[nbins] = _build(nbins)
    nc = _CACHE[nbins]
    trace = bool(os.environ.get("KERNEL_TRACE"))
    res = run_bass_kernel_spmd(nc, in_maps, core_ids=list(range(NCORES)),
                               trace=trace)
    global LAST_RESULT
    LAST_RESULT = res
    outs = res.results
    emb = np.concatenate([outs[k]["emb_o"] for k in range(NCORES)], 0)
    dec = np.concatenate([outs[k]["dec_o"] for k in range(NCORES)], 0)
    lg = np.concatenate([outs[k]["lg_o"] for k in range(NCORES)], 0)
    lga = np.concatenate([outs[k]["lga_o"] for k in range(NCORES)], 0)
    return emb, dec, lg, lga


def bench(iters=8, **inputs):
    """Device-resident repeated execution timing (ns, min over iters)."""
    import time as _time
    import jax
    from jax.sharding import Mesh, PartitionSpec, NamedSharding
    from jax.experimental.shard_map import shard_map
    from concourse import bass2jax

    in_maps, nbins = _make_in_maps(inputs)
    if nbins not in _CACHE:
        _CACHE[nbins] = _build(nbins)
    nc = _CACHE[nbins]
    bass2jax.install_neuronx_cc_hook()
    pname = nc.partition_id_tensor.name if nc.partition_id_tensor else None
    in_names, out_names, out_avals, zeros = [], [], [], []
    for alloc in nc.m.functions[0].allocations:
        if not isinstance(alloc, mybir.MemoryLocationSet):
            continue
        name = alloc.memorylocations[0].name
        if alloc.kind == "ExternalInput":
            if name != pname:
                in_names.append(name)
        elif alloc.kind == "ExternalOutput":
            out_names.append(name)
            shape = tuple(alloc.tensor_shape)
            dtype = mybir.dt.np(alloc.dtype)
            out_avals.append(jax.core.ShapedArray(shape, dtype))
            zeros.append(np.zeros(shape, dtype))
    n_params = len(in_names)
    all_names = in_names + out_names + ([pname] if pname else [])

    def _body(*args):
        ops = list(args)
        if pname:
            ops.append(bass2jax.partition_id_tensor())
        return tuple(bass2jax._bass_exec_p.bind(
            *ops, out_avals=tuple(out_avals), in_names=tuple(all_names),
            out_names=tuple(out_names), lowering_input_output_aliases=(),
            sim_require_finite=True, sim_require_nnan=True, nc=nc))

    devices = jax.devices()[:NCORES]
    mesh = Mesh(np.asarray(devices), ("core",))
    nio = n_params + len(out_avals)
    fn = jax.jit(shard_map(_body, mesh=mesh,
                           in_specs=(PartitionSpec("core"),) * nio,
                           out_specs=(PartitionSpec("core"),) * len(out_avals),
                           check_rep=False), keep_unused=True)
    per_core = [[np.asarray(m[nm]) for nm in in_names] for m in in_maps]
    cat = [np.concatenate([per_core[c][i] for c in range(NCORES)], axis=0)
           for i in range(n_params)]
    catz = [np.zeros((NCORES * z.shape[0], *z.shape[1:]), z.dtype)
            for z in zeros]
    sh = NamedSharding(mesh, PartitionSpec("core"))
    din_ = [jax.device_put(x, sh) for x in cat]
    dz = [jax.device_put(x, sh) for x in catz]
    r = fn(*din_, *dz)
    jax.block_until_ready(r)
    times = []
    for _ in range(iters):
        t0 = _time.perf_counter()
        r = fn(*din_, *dz)
        jax.block_until_ready(r)
        times.append((_time.perf_counter() - t0) * 1e9)
    outs = [np.asarray(r[i]).reshape(NCORES, *out_avals[i].shape)
            for i in range(len(out_names))]
    res = {nm: outs[i].reshape(-1, *out_avals[i].shape[1:])
           for i, nm in enumerate(out_names)}
    return res, min(times)


# revision 25
# speedup vs baseline: 1.6101x; 1.6101x over previous
"""Trainium2 Bass kernel for nn_Encoder_67980742361889 (GAT + 2-layer
transformer encoder, dual-branch DGI-style head), SPMD across 8 NeuronCores.

Sharding: nodes (rows) are split 375/core.  GAT edges are sharded by dst,
bin-packed so each 128-edge bin holds only whole dst-segments; duplicate
edges merge into multiplicities (identical logits => exp log-bias).
Weights are replicated; K/V (and the xl table / final embeddings) are
exchanged with AllGather collectives.  Matmuls run in bf16 (fp32 PSUM).

Self-contained: call kernel(**inputs) with the full-size inputs; returns the
full (embedding, decoded, logits, logits_a) tuple.
"""
import os
import sys

sys.path.insert(0, "/opt/trn_rl_repo")

from contextlib import ExitStack

import numpy as np
import ml_dtypes

import concourse.bass as bass
import concourse.tile as tile
from concourse import mybir
from concourse import bacc
from concourse.bass_utils import run_bass_kernel_spmd
from concourse.masks import make_identity

# ---------------------------------------------------------------- constants
N = 3000
F_IN = 3000
D = 512
H = 8
CH = 64
DFF = 2048
NCORES = 8
R = N // NCORES            # 375 rows / core
BIN = 128
SLOTS = 16
LRELU_GAT = 0.2
LRELU_ACT = 0.01
NEG_BIG = -1.0e30

FP = mybir.dt.float32
BF = mybir.dt.bfloat16
I32 = mybir.dt.int32
AXX = mybir.AxisListType.X
AF = mybir.ActivationFunctionType
ALU = mybir.AluOpType

KCH = [128] * 23 + [56]            # 3000 contraction chunks
MCH = [128, 128, 119]              # 375 row chunks
BF_NP = ml_dtypes.bfloat16

_CACHE = {}
LAST_RESULT = None


# ------------------------------------------------------------- host edge prep
def _prep_edges(edge_index):
    src = np.asarray(edge_index[0], dtype=np.int64)
    dst = np.asarray(edge_index[1], dtype=np.int64)
    loops = np.arange(N, dtype=np.int64)
    src = np.concatenate([src, loops])
    dst = np.concatenate([dst, loops])
    key = dst * N + src
    uniq, counts = np.unique(key, return_counts=True)
    u_dst = (uniq // N).astype(np.int64)
    u_src = (uniq % N).astype(np.int64)
    per_core = []
    for k in range(NCORES):
        lo = k * R
        m = (u_dst >= lo) & (u_dst < lo + R)
        s_src = u_src[m]
        s_dst = u_dst[m] - lo
        s_mult = counts[m].astype(np.float64)
        order = np.argsort(s_dst, kind="stable")
        s_src, s_dst, s_mult = s_src[order], s_dst[order], s_mult[order]
        segs = []
        start = 0
        for i in range(1, len(s_dst) + 1):
            if i == len(s_dst) or s_dst[i] != s_dst[start]:
                segs.append((int(s_dst[start]), start, i))
                start = i
        bins = []
        cur, cur_e = [], 0
        for seg in segs:
            ln = seg[2] - seg[1]
            assert ln <= BIN
            if cur and (cur_e + ln > BIN or len(cur) >= SLOTS):
                bins.append(cur)
                cur, cur_e = [], 0
            cur.append(seg)
            cur_e += ln
        if cur:
            bins.append(cur)
        per_core.append((bins, s_src, s_mult))
    nbins = max(len(b) for b, _, _ in per_core)

    out = []
    for k in range(NCORES):
        bins, s_src, s_mult = per_core[k]
        Q = np.zeros((nbins, BIN, SLOTS), np.float32)
        QT = np.zeros((nbins, SLOTS, BIN), np.float32)
        srcg = np.zeros((nbins, BIN, 1), np.int32)
        slotd = np.full((nbins, SLOTS, 1), R, np.int32)     # pad -> trash row R
        logm = np.full((nbins, BIN, 1), NEG_BIG, np.float32)
        for b, seg_list in enumerate(bins):
            e = 0
            for s, (dloc, i0, i1) in enumerate(seg_list):
                slotd[b, s, 0] = dloc
                for i in range(i0, i1):
                    Q[b, e, s] = 1.0
                    QT[b, s, e] = 1.0
                    srcg[b, e, 0] = s_src[i]
                    logm[b, e, 0] = np.log(s_mult[i])
                    e += 1
        nkc = (nbins * SLOTS + 127) // 128
        Ep = np.zeros((nbins, 3, 128, BIN), np.float32)
        Pt = np.zeros((nkc, 128, 384), np.float32)
        for b in range(nbins):
            for e in range(BIN):
                s = int(np.argmax(Q[b, e])) if Q[b, e].any() else -1
                if s >= 0:
                    dl = int(slotd[b, s, 0])
                    Ep[b, dl // 128, dl % 128, e] = 1.0
        for b in range(nbins):
            for s in range(SLOTS):
                dl = int(slotd[b, s, 0])
                if dl < R:
                    gs = b * SLOTS + s
                    Pt[gs // 128, gs % 128, dl] = 1.0
        out.append(dict(Q=Q.astype(BF_NP), srcg=srcg, logm=logm,
                        Ep=Ep.astype(BF_NP), Pt=Pt.astype(BF_NP)))
    return out, nbins


# ----------------------------------------------------------------- builder
def _build(nbins):
    nc = bacc.Bacc(target_bir_lowering=False)
    NKC = (nbins * SLOTS + 127) // 128

    def din(name, shape, dt=BF):
        return nc.declare_dram_parameter(name, list(shape), dt, isOutput=False)[:]

    def dout(name, shape, dt=FP):
        return nc.declare_dram_parameter(name, list(shape), dt, isOutput=True)[:]

    xT = {0: din("xTa", [F_IN, R]), 1: din("xTb", [F_IN, R])}
    adjT = din("adjT", [N, R])
    rsr = din("rsrecip", [R, 1], FP)
    Wl = din("Wl", [F_IN, D]); Wr = din("Wr", [F_IN, D]); Wp = din("Wproj", [F_IN, D])
    WqkvT = din("WqkvT", [2, D, 3 * D])
    bqkvT = din("bqkvT", [2, 3 * D, 1], FP)
    WoT = din("WoT", [2, D, D])
    Wff1 = din("Wff1", [2, D, DFF])
    bff1T = din("bff1T", [2, DFF, 1], FP)
    Wff2 = din("Wff2", [2, DFF, D])
    Wout = din("Wout", [D, F_IN])
    WdT = din("WdiscT", [D, D])
    rows = din("rows", [1, 17 * D + F_IN])        # bf16, packing per _pack_rows
    sca = din("sca", [1, 4], FP)                  # a, 1-a, bdisc, 0
    Qm = din("Qm", [nbins, BIN, SLOTS])
    Epm = din("Epm", [nbins, 3, 128, BIN])
    Ptm = din("Ptm", [NKC, 128, 384])
    srcA = din("srcA", [BIN, nbins], I32)
    srcB = din("srcB", [BIN, nbins], I32)
    logm = din("logm", [nbins, BIN, 1], FP)

    emb_o = dout("emb_o", [R, D])
    dec_o = dout("dec_o", [R, F_IN])
    lg_o = dout("lg_o", [R, 2])
    lga_o = dout("lga_o", [R, 2])

    RG = [list(range(NCORES))]

    def bcast(ap2d, parts):
        return bass.AP(tensor=ap2d.tensor, offset=ap2d.offset,
                       ap=[[0, parts]] + [list(x) for x in ap2d.ap[1:]])

    with tile.TileContext(nc) as tc, ExitStack() as ctx:
        dram = ctx.enter_context(tc.tile_pool(name="dram", bufs=1, space="DRAM"))
        const = ctx.enter_context(tc.tile_pool(name="const", bufs=1))
        persist = ctx.enter_context(tc.tile_pool(name="persist", bufs=1))

        # DRAM internals
        xl_bounce = {br: dram.tile([R, D], BF, name=f"xlb{br}") for br in (0, 1)}
        xl_ag = {br: dram.tile([NCORES, R, D], BF, addr_space="Shared",
                               name=f"xlag{br}") for br in (0, 1)}
        xl_flat = {br: xl_ag[br].rearrange("a c d -> (a c) d") for br in (0, 1)}
        gat_stage = dram.tile([2, NKC * 128, D], BF)
        kv_bounce = {}
        kv_ag = {}
        for l in range(2):
            for br in range(2):
                kv_bounce[l, br] = dram.tile([2 * D * R], BF, name=f"kvb{l}{br}")
                kv_ag[l, br] = dram.tile([NCORES, 2 * D * R], BF,
                                         addr_space="Shared", name=f"kvg{l}{br}")
        emb_bounce = dram.tile([2, R, D], BF)
        emb_ag = dram.tile([NCORES, 2, R, D], BF, addr_space="Shared")

        # constants
        ident = const.tile([128, 128], BF)
        make_identity(nc, ident[:])
        row_specs = [
            ("bproj", D), ("att", D), ("gatb", D),
            ("bv0", D), ("bv1", D), ("bo0", D), ("bo1", D),
            ("bf2_0", D), ("bf2_1", D),
            ("l1g0", D), ("l1b0", D), ("l2g0", D), ("l2b0", D),
            ("l1g1", D), ("l1b1", D), ("l2g1", D), ("l2b1", D),
        ]
        off = 0
        row_tiles = {}
        for nm, w in row_specs:
            t = const.tile([128, w], BF, name=f"row_{nm}")
            nc.sync.dma_start(out=t[:], in_=bcast(rows[:, off:off + w], 128))
            row_tiles[nm] = t
            off += w
        bout_off = off
        sca_t = const.tile([128, 4], FP)
        nc.sync.dma_start(out=sca_t[:], in_=bcast(sca[:, :], 128))
        tail = NKC * 128 - nbins * SLOTS
        if tail > 0:
            zt = const.tile([128, D], BF)
            nc.vector.memset(zt[:], 0.0)
            for br in (0, 1):
                nc.sync.dma_start(
                    out=gat_stage[br, nbins * SLOTS:NKC * 128, :],
                    in_=zt[:tail, :])

        # persistent activations (bf16)
        x_res = {br: [persist.tile([128, D], BF, name=f"xres{br}{m}", bufs=1)
                      for m in range(3)] for br in (0, 1)}
        emb_bf = {br: [persist.tile([128, D], BF, name=f"emb{br}{m}", bufs=1)
                       for m in range(3)] for br in (0, 1)}
        comb_A = [persist.tile([128, D], BF, name=f"combA{m}", bufs=1)
                  for m in range(3)]
        g_ro = {br: [persist.tile([128, D], BF, name=f"gro{br}{m}", bufs=1)
                     for m in range(3)] for br in (0, 1)}
        xr_sb = {br: [persist.tile([128, D], BF, name=f"xrsb{br}{m}", bufs=1)
                      for m in range(3)] for br in (0, 1)}

        # ============================================================ P1
        with tc.tile_pool(name="p1x", bufs=1) as p1x, \
             tc.tile_pool(name="p1w", bufs=4) as p1w, \
             tc.tile_pool(name="p1o", bufs=4) as p1o, \
             tc.tile_pool(name="p1ps", bufs=1, space="PSUM") as p1ps:
            xt_tiles = {}
            for br in (0, 1):
                for ki, kw in enumerate(KCH):
                    t = p1x.tile([128, R], BF, name=f"p1x{br}_{ki}", bufs=1)
                    nc.sync.dma_start(out=t[:kw, :],
                                      in_=xT[br][ki * 128:ki * 128 + kw, :])
                    xt_tiles[br, ki] = t

            def proj_pass(W, consume):
                ps = {(br, m): p1ps.tile([128, D], FP, name=f"p1ps{br}{m}",
                                         tag=f"p1ps{br}{m}", bufs=1)
                      for br in (0, 1) for m in range(3)}
                for ki, kw in enumerate(KCH):
                    wt = p1w.tile([128, D], BF, tag="p1w", name="wt")
                    nc.sync.dma_start(out=wt[:kw, :], in_=W[ki * 128:ki * 128 + kw, :])
                    for br in (0, 1):
                        mo = 0
                        for m, mw in enumerate(MCH):
                            nc.tensor.matmul(
                                out=ps[br, m][:mw, :],
                                lhsT=xt_tiles[br, ki][:kw, mo:mo + mw],
                                rhs=wt[:kw, :],
                                start=(ki == 0), stop=(ki == len(KCH) - 1))
                            mo += mw
                for br in (0, 1):
                    for m, mw in enumerate(MCH):
                        consume(br, m, mw, ps[br, m])

            def use_xl(br, m, mw, ps):
                t = p1o.tile([128, D], BF, tag="p1o", name="t")
                nc.vector.tensor_copy(out=t[:mw, :], in_=ps[:mw, :])
                nc.sync.dma_start(out=xl_bounce[br][m * 128:m * 128 + mw, :],
                                  in_=t[:mw, :])
            proj_pass(Wl, use_xl)
            for br in (0, 1):
                nc.gpsimd.collective_compute(
                    "AllGather", ALU.bypass, replica_groups=RG,
                    ins=[xl_bounce[br][:].opt()], outs=[xl_ag[br][:].opt()])

            def use_xp(br, m, mw, ps):
                nc.vector.tensor_tensor(out=x_res[br][m][:mw, :], in0=ps[:mw, :],
                                        in1=row_tiles["bproj"][:mw, :], op=ALU.add)
            proj_pass(Wp, use_xp)

            def use_xr(br, m, mw, ps):
                nc.vector.tensor_copy(out=xr_sb[br][m][:mw, :], in_=ps[:mw, :])
            proj_pass(Wr, use_xr)

        # ============================================================ P2+P3
        def layernorm(pool, big, x_in, mw, g_row, b_row, out_bf):
            st = pool.tile([128, 6], FP, tag="ln_st", name="st")
            nc.vector.bn_stats(out=st[:mw, :], in_=x_in[:mw, :])
            mv = pool.tile([128, 2], FP, tag="ln_mv", name="mv")
            nc.vector.bn_aggr(out=mv[:mw, :], in_=st[:mw, :])
            tv = pool.tile([128, 1], FP, tag="ln_tv", name="tv")
            nc.vector.tensor_scalar_add(out=tv[:mw, :], in0=mv[:mw, 1:2],
                                        scalar1=1e-5)
            sd = pool.tile([128, 1], FP, tag="ln_sd", name="sd")
            nc.scalar.sqrt(out=sd[:mw, :], in_=tv[:mw, :])
            rstd = pool.tile([128, 1], FP, tag="ln_rs", name="rstd")
            nc.vector.reciprocal(out=rstd[:mw, :], in_=sd[:mw, :])
            xc = big.tile([128, D], FP, tag="scr", name="xc")
            nc.vector.scalar_tensor_tensor(
                out=xc[:mw, :], in0=x_in[:mw, :], scalar=mv[:mw, 0:1],
                in1=g_row[:mw, :], op0=ALU.subtract, op1=ALU.mult)
            nc.vector.scalar_tensor_tensor(
                out=out_bf[:mw, :], in0=xc[:mw, :], scalar=rstd[:mw, :],
                in1=b_row[:mw, :], op0=ALU.mult, op1=ALU.add)

        with tc.tile_pool(name="g_sb", bufs=2) as gsb, \
             tc.tile_pool(name="g_eb", bufs=4) as geb, \
             tc.tile_pool(name="g_idx", bufs=3) as gidx, \
             tc.tile_pool(name="t_one", bufs=1) as tone, \
             tc.tile_pool(name="t_w", bufs=1) as twr, \
             tc.tile_pool(name="t_ws", bufs=4) as tws, \
             tc.tile_pool(name="t_kt", bufs=1) as tkt, \
             tc.tile_pool(name="t_vs", bufs=1) as tvs, \
             tc.tile_pool(name="t_sc", bufs=4) as tsc, \
             tc.tile_pool(name="t_sm", bufs=2) as tsm, \
             tc.tile_pool(name="t_ps", bufs=1, space="PSUM") as tps:

            # ---------------- GAT bins (scheduler interleaves with P3)
            sidx_all = {}
            for br in (0, 1):
                t = gidx.tile([BIN, nbins], I32, name=f"sidxall{br}", bufs=1)
                nc.sync.dma_start(out=t[:], in_=(srcA if br == 0 else srcB))
                sidx_all[br] = t
            for br in (0, 1):
                for b in range(nbins):
                    lm = gidx.tile([BIN, 1], FP, tag="lm", name="lm")
                    nc.sync.dma_start(out=lm[:], in_=logm[b])
                    qb = gsb.tile([BIN, SLOTS], BF, tag="qb", name="qb")
                    nc.sync.dma_start(out=qb[:], in_=Qm[b])
                    ept = gsb.tile([128, 3, BIN], BF, tag="ept", name="ept")
                    nc.sync.dma_start(out=ept[:],
                                      in_=Epm[b].rearrange("kc p e -> p kc e"))
                    xlg = gsb.tile([BIN, D], BF, tag="xlg", name="xlg", bufs=4)
                    nc.gpsimd.indirect_dma_start(
                        out=xlg[:], out_offset=None,
                        in_=xl_flat[br],
                        in_offset=bass.IndirectOffsetOnAxis(
                            ap=sidx_all[br][:, b:b + 1], axis=0))

                    ps_z = tps.tile([BIN, D], FP, tag="gp", name="ps_z", bufs=2)
                    for kc, kw in enumerate(MCH):
                        nc.tensor.matmul(out=ps_z[:], lhsT=ept[:kw, kc, :],
                                         rhs=xr_sb[br][kc][:kw, :],
                                         start=(kc == 0), stop=(kc == 2))
                    z = geb.tile([BIN, D], BF, tag="ebuf", name="z")
                    nc.vector.tensor_tensor(out=z[:], in0=ps_z[:], in1=xlg[:],
                                            op=ALU.add)
                    lr = geb.tile([BIN, D], BF, tag="ebuf", name="lr")
                    nc.vector.scalar_tensor_tensor(
                        out=lr[:], in0=z[:], scalar=LRELU_GAT, in1=z[:],
                        op0=ALU.mult, op1=ALU.max)
                    tm = geb.tile([BIN, D], BF, tag="ebuf", name="tm")
                    nc.vector.tensor_tensor(out=tm[:], in0=lr[:],
                                            in1=row_tiles["att"][:], op=ALU.mult)
                    lgt = gidx.tile([BIN, H], FP, tag="lgt", name="lgt")
                    nc.vector.reduce_sum(
                        out=lgt[:, :, None],
                        in_=tm[:].rearrange("p (h c) -> p h c", h=H), axis=AXX)
                    w = gidx.tile([BIN, H], BF, tag="w", name="w")
                    nc.scalar.activation(out=w[:], in_=lgt[:], func=AF.Exp,
                                         bias=lm[:, :1], scale=1.0)
                    v = geb.tile([BIN, D], BF, tag="ebuf", name="v")
                    nc.vector.tensor_tensor(
                        out=v[:].rearrange("p (h c) -> p h c", h=H),
                        in0=xlg[:].rearrange("p (h c) -> p h c", h=H),
                        in1=w[:].to_broadcast([BIN, H, CH]), op=ALU.mult)
                    ps_u = tps.tile([SLOTS, D], FP, tag="gp", name="ps_u", bufs=2)
                    nc.tensor.matmul(out=ps_u[:], lhsT=qb[:], rhs=v[:],
                                     start=True, stop=True)
                    ps_s = tps.tile([SLOTS, H], FP, tag="gp", name="ps_s", bufs=2)
                    nc.tensor.matmul(out=ps_s[:], lhsT=qb[:], rhs=w[:],
                                     start=True, stop=True)
                    sse = gidx.tile([SLOTS, H], FP, tag="sse", name="sse")
                    nc.vector.tensor_scalar_add(out=sse[:], in0=ps_s[:],
                                                scalar1=1e-30)
                    rq = gidx.tile([SLOTS, H], FP, tag="rq", name="rq")
                    nc.vector.reciprocal(out=rq[:], in_=sse[:])
                    outr = gsb.tile([SLOTS, D], BF, tag="outr", name="outr")
                    nc.vector.tensor_tensor(
                        out=outr[:].rearrange("p (h c) -> p h c", h=H),
                        in0=ps_u[:].rearrange("p (h c) -> p h c", h=H),
                        in1=rq[:].to_broadcast([SLOTS, H, CH]), op=ALU.mult)
                    nc.sync.dma_start(
                        out=gat_stage[br, b * SLOTS:(b + 1) * SLOTS, :],
                        in_=outr[:])

            # ---------------- transformer
            tT = [tone.tile([128, R], BF, name=f"tT{f}", bufs=1) for f in range(4)]
            qT_t = {br: [tone.tile([128, R], BF, name=f"qT{br}_{f}", bufs=1)
                         for f in range(4)] for br in (0, 1)}
            oT_t = [tone.tile([128, R], BF, name=f"oT_t{f}", bufs=1)
                    for f in range(4)]
            kT_t = [tone.tile([128, R], BF, name=f"kT_t{f}", bufs=1)
                    for f in range(4)]
            rT = [tone.tile([128, R], BF, name=f"rT{f}", bufs=1) for f in range(8)]

            def transpose_rows(psp, src_tiles, dst_tiles, pbufs=3):
                mo = 0
                for m, mw in enumerate(MCH):
                    for f in range(4):
                        pt = psp.tile([128, 128], BF, tag="pb", name="pt",
                                      bufs=pbufs)
                        nc.tensor.transpose(
                            out=pt[:, :mw],
                            in_=src_tiles[m][:mw, f * 128:(f + 1) * 128],
                            identity=ident[:mw, :mw])
                        nc.vector.tensor_copy(out=dst_tiles[f][:, mo:mo + mw],
                                              in_=pt[:, :mw])
                    mo += mw

            for l in range(2):
                wq4 = [twr.tile([128, 3 * D], BF, name=f"wq{dk}", tag=f"wq{dk}",
                                bufs=1) for dk in range(4)]
                for dk in range(4):
                    nc.sync.dma_start(out=wq4[dk][:],
                                      in_=WqkvT[l, dk * 128:(dk + 1) * 128, :])
                wo4 = [twr.tile([128, D], BF, name=f"wo{dk}", tag=f"wo{dk}",
                                bufs=1) for dk in range(4)]
                for dk in range(4):
                    nc.sync.dma_start(out=wo4[dk][:],
                                      in_=WoT[l, dk * 128:(dk + 1) * 128, :])
                wf1 = [twr.tile([128, DFF], BF, name=f"wf1_{dk}", tag=f"wf1_{dk}",
                                bufs=1) for dk in range(4)]
                for dk in range(4):
                    nc.sync.dma_start(out=wf1[dk][:],
                                      in_=Wff1[l, dk * 128:(dk + 1) * 128, :])
                wf2 = [twr.tile([128, D], BF, name=f"wf2_{c}", tag=f"wf2_{c}",
                                bufs=1) for c in range(16)]
                for c in range(16):
                    nc.sync.dma_start(out=wf2[c][:],
                                      in_=Wff2[l, c * 128:(c + 1) * 128, :])
                bq_sl = {}
                for part in range(3):
                    for f in range(4):
                        t = tsm.tile([128, 1], FP, tag=f"bq{part}{f}",
                                     name="t", bufs=1)
                        nc.sync.dma_start(
                            out=t[:],
                            in_=bqkvT[l, part * D + f * 128:
                                      part * D + (f + 1) * 128, :])
                        bq_sl[part, f] = t
                bf1_sl = {}
                for c in range(16):
                    t = tsm.tile([128, 1], FP, tag=f"bf1{c}", name="t", bufs=1)
                    nc.sync.dma_start(out=t[:],
                                      in_=bff1T[l, c * 128:(c + 1) * 128, :])
                    bf1_sl[c] = t

                # QKV + AG for both branches first (hides AG latency)
                for br in (0, 1):
                    transpose_rows(tps, x_res[br], tT)
                    for part, dest in ((0, qT_t[br]), (1, kT_t)):
                        for f in range(4):
                            ps = tps.tile([128, R], FP, tag="pb", name="ps",
                                          bufs=3)
                            for dk in range(4):
                                nc.tensor.matmul(
                                    out=ps[:],
                                    lhsT=wq4[dk][:, part * D + f * 128:
                                                 part * D + (f + 1) * 128],
                                    rhs=tT[dk][:],
                                    start=(dk == 0), stop=(dk == 3))
                            nc.vector.tensor_scalar_add(
                                out=dest[f][:], in0=ps[:],
                                scalar1=bq_sl[part, f][:, :1])
                    kvb = kv_bounce[l, br]
                    ktv = kvb.rearrange("(f t) -> f t", f=2 * D)
                    for f in range(4):
                        nc.sync.dma_start(out=ktv[f * 128:(f + 1) * 128, :],
                                          in_=kT_t[f][:])
                    mo = 0
                    for m, mw in enumerate(MCH):
                        ps = tps.tile([128, D], FP, tag="pb", name="ps", bufs=3)
                        for dk in range(4):
                            nc.tensor.matmul(
                                out=ps[:mw, :], lhsT=tT[dk][:, mo:mo + mw],
                                rhs=wq4[dk][:, 2 * D:3 * D],
                                start=(dk == 0), stop=(dk == 3))
                        vt = tsc.tile([128, D], BF, tag="scb", name="vt")
                        nc.vector.tensor_tensor(
                            out=vt[:mw, :], in0=ps[:mw, :],
                            in1=row_tiles[f"bv{l}"][:mw, :], op=ALU.add)
                        vv = kvb[D * R:].rearrange("(t f) -> t f", t=R)
                        nc.sync.dma_start(out=vv[mo:mo + mw, :], in_=vt[:mw, :])
                        mo += mw
                    nc.gpsimd.collective_compute(
                        "AllGather", ALU.bypass, replica_groups=RG,
                        ins=[kvb[:].opt()], outs=[kv_ag[l, br][:].opt()])

                # attention + oproj + LN1 per branch
                for br in (0, 1):
                    kvg = kv_ag[l, br]
                    kt4 = []
                    for blk in range(NCORES):
                        t = tkt.tile([128, 4, R], BF, name=f"kt4_{blk}",
                                     tag=f"kt4_{blk}", bufs=1)
                        nc.sync.dma_start(
                            out=t[:],
                            in_=kvg[blk, :D * R].rearrange(
                                "(c p t) -> p c t", c=4, p=128))
                        kt4.append(t)
                    vsb = []
                    for tb in range(24):
                        blk, sub = tb // 3, tb % 3
                        toff, tsubw = sub * 128, MCH[sub]
                        t = tvs.tile([128, H, CH + 1], BF, name=f"vsb{tb}",
                                     tag=f"vsb{tb}", bufs=1)
                        nc.vector.memset(t[:, :, CH:CH + 1], 1.0)
                        vv = kvg[blk, D * R:].rearrange("(t f) -> t f", t=R)
                        nc.sync.dma_start(
                            out=t[:tsubw, :, :CH],
                            in_=vv[toff:toff + tsubw, :].rearrange(
                                "t (h c) -> t h c", h=H))
                        vsb.append(t)
                    for h in range(H):
                        ps_o = tps.tile([65, R], FP, tag="po", name="ps_o",
                                        bufs=1)
                        for tb in range(24):
                            blk, sub = tb // 3, tb % 3
                            toff, tsubw = sub * 128, MCH[sub]
                            ps_s = tps.tile([128, R], FP, tag="psx",
                                            name="ps_s", bufs=2)
                            nc.tensor.matmul(
                                out=ps_s[:tsubw, :],
                                lhsT=kt4[blk][(h % 2) * 64:(h % 2) * 64 + 64,
                                              h // 2, toff:toff + tsubw],
                                rhs=qT_t[br][h // 2][(h % 2) * 64:
                                                     (h % 2) * 64 + 64, :],
                                start=True, stop=True)
                            eT = tsc.tile([128, R], BF, tag="eT", name="eT",
                                          bufs=2)
                            nc.scalar.activation(out=eT[:tsubw, :],
                                                 in_=ps_s[:tsubw, :],
                                                 func=AF.Exp, scale=0.125)
                            nc.tensor.matmul(
                                out=ps_o[:],
                                lhsT=vsb[tb][:tsubw, h, :],
                                rhs=eT[:tsubw, :],
                                start=(tb == 0), stop=(tb == 23))
                        rd = tsm.tile([1, R], BF, tag="rd", name="rd", bufs=2)
                        with nc.allow_low_precision(reason="softmax recip bf16"):
                            nc.vector.reciprocal(out=rd[:], in_=ps_o[64:65, :])
                        rb = tsm.tile([64, R], BF, tag="rb", name="rb", bufs=2)
                        nc.gpsimd.partition_broadcast(rb[:], rd[:])
                        nc.vector.tensor_tensor(
                            out=oT_t[h // 2][(h % 2) * 64:(h % 2) * 64 + 64, :],
                            in0=ps_o[:64, :], in1=rb[:], op=ALU.mult)
                    mo = 0
                    for m, mw in enumerate(MCH):
                        ps = tps.tile([128, D], FP, tag="pb", name="ps", bufs=3)
                        for dk in range(4):
                            nc.tensor.matmul(out=ps[:mw, :],
                                             lhsT=oT_t[dk][:, mo:mo + mw],
                                             rhs=wo4[dk][:],
                                             start=(dk == 0), stop=(dk == 3))
                        e2 = tsc.tile([128, D], FP, tag="scr", name="e2")
                        nc.vector.tensor_tensor(out=e2[:mw, :], in0=ps[:mw, :],
                                                in1=row_tiles[f"bo{l}"][:mw, :],
                                                op=ALU.add)
                        e3 = tsc.tile([128, D], FP, tag="scr", name="e3")
                        nc.vector.tensor_tensor(out=e3[:mw, :], in0=e2[:mw, :],
                                                in1=x_res[br][m][:mw, :],
                                                op=ALU.add)
                        layernorm(tsm, tsc, e3, mw, row_tiles[f"l1g{l}"],
                                  row_tiles[f"l1b{l}"], x_res[br][m])
                        mo += mw
                # FFN per branch (rT halves to bound SBUF)
                for br in (0, 1):
                    transpose_rows(tps, x_res[br], tT)
                    ffa = [tsc.tile([128, D], BF, tag=f"ffa{m}", name="ffa",
                                    bufs=1) for m in range(3)]
                    for half in range(2):
                        for ci in range(8):
                            c = half * 8 + ci
                            ps = tps.tile([128, R], FP, tag="pb", name="ps",
                                          bufs=3)
                            for dk in range(4):
                                nc.tensor.matmul(
                                    out=ps[:],
                                    lhsT=wf1[dk][:, c * 128:(c + 1) * 128],
                                    rhs=tT[dk][:],
                                    start=(dk == 0), stop=(dk == 3))
                            nc.scalar.activation(out=rT[ci][:], in_=ps[:],
                                                 func=AF.Relu,
                                                 bias=bf1_sl[c][:, :1], scale=1.0)
                        mo = 0
                        for m, mw in enumerate(MCH):
                            ps = tps.tile([128, D], FP, tag="pb", name="ps",
                                          bufs=3)
                            for ci in range(8):
                                nc.tensor.matmul(
                                    out=ps[:mw, :],
                                    lhsT=rT[ci][:, mo:mo + mw],
                                    rhs=wf2[half * 8 + ci][:],
                                    start=(ci == 0), stop=(ci == 7))
                            if half == 0:
                                nc.vector.tensor_copy(out=ffa[m][:mw, :],
                                                      in_=ps[:mw, :])
                            else:
                                e2 = tsc.tile([128, D], FP, tag="scr", name="e2")
                                nc.vector.tensor_tensor(
                                    out=e2[:mw, :], in0=ps[:mw, :],
                                    in1=ffa[m][:mw, :], op=ALU.add)
                                e2b = tsc.tile([128, D], FP, tag="scr",
                                               name="e2b")
                                nc.vector.tensor_tensor(
                                    out=e2b[:mw, :], in0=e2[:mw, :],
                                    in1=row_tiles[f"bf2_{l}"][:mw, :], op=ALU.add)
                                e3 = tsc.tile([128, D], FP, tag="scr", name="e3")
                                nc.vector.tensor_tensor(
                                    out=e3[:mw, :], in0=e2b[:mw, :],
                                    in1=x_res[br][m][:mw, :], op=ALU.add)
                                layernorm(tsm, tsc, e3, mw,
                                          row_tiles[f"l2g{l}"],
                                          row_tiles[f"l2b{l}"], x_res[br][m])
                            mo += mw

            # ==================================================== P4 comb
            for br in (0, 1):
                mo = 0
                for m, mw in enumerate(MCH):
                    pg = tps.tile([128, D], FP, tag="pb", name="pg", bufs=3)
                    for kc in range(NKC):
                        gs = tsc.tile([128, D], BF, tag="scb", name="gs")
                        nc.sync.dma_start(
                            out=gs[:],
                            in_=gat_stage[br, kc * 128:(kc + 1) * 128, :])
                        ptk = tws.tile([128, 384], BF, tag="w", name="ptk")
                        nc.sync.dma_start(out=ptk[:, :384], in_=Ptm[kc])
                        nc.tensor.matmul(out=pg[:mw, :],
                                         lhsT=ptk[:, mo:mo + mw],
                                         rhs=gs[:],
                                         start=(kc == 0), stop=(kc == NKC - 1))
                    g1 = tsc.tile([128, D], FP, tag="scr", name="g1")
                    nc.vector.tensor_tensor(out=g1[:mw, :], in0=pg[:mw, :],
                                            in1=row_tiles["gatb"][:mw, :],
                                            op=ALU.add)
                    xs = tsc.tile([128, D], FP, tag="scr", name="xs")
                    nc.vector.tensor_scalar_mul(out=xs[:mw, :],
                                                in0=x_res[br][m][:mw, :],
                                                scalar1=sca_t[:mw, 1:2])
                    cmb = tsc.tile([128, D], FP, tag="scr", name="cmb")
                    nc.vector.scalar_tensor_tensor(
                        out=cmb[:mw, :], in0=g1[:mw, :],
                        scalar=sca_t[:mw, 0:1],
                        in1=xs[:mw, :], op0=ALU.mult, op1=ALU.add)
                    embf = tsc.tile([128, D], FP, tag="scr", name="embf")
                    nc.vector.scalar_tensor_tensor(
                        out=embf[:mw, :], in0=cmb[:mw, :], scalar=LRELU_ACT,
                        in1=cmb[:mw, :], op0=ALU.mult, op1=ALU.max)
                    nc.vector.tensor_copy(out=emb_bf[br][m][:mw, :],
                                          in_=embf[:mw, :])
                    if br == 0:
                        nc.vector.tensor_copy(out=comb_A[m][:mw, :],
                                              in_=cmb[:mw, :])
                        nc.sync.dma_start(out=emb_o[mo:mo + mw, :],
                                          in_=embf[:mw, :])
                    nc.sync.dma_start(out=emb_bounce[br, mo:mo + mw, :],
                                      in_=emb_bf[br][m][:mw, :])
                    mo += mw
            nc.gpsimd.collective_compute(
                "AllGather", ALU.bypass, replica_groups=RG,
                ins=[emb_bounce[:].opt()], outs=[emb_ag[:].opt()])

            # ==================================================== P5 dec
            transpose_rows(tps, comb_A, tT)
            for nchi in range(6):
                wo_t = []
                for f in range(4):
                    t = tws.tile([128, 500], BF, tag="w", name="wt")
                    nc.sync.dma_start(
                        out=t[:],
                        in_=Wout[f * 128:(f + 1) * 128,
                                 nchi * 500:(nchi + 1) * 500])
                    wo_t.append(t)
                bo_sl = tsc.tile([128, 500], BF, tag="scb", name="bo_sl")
                nc.sync.dma_start(
                    out=bo_sl[:],
                    in_=bcast(rows[:, bout_off + nchi * 500:
                                   bout_off + (nchi + 1) * 500], 128))
                mo = 0
                for m, mw in enumerate(MCH):
                    ps = tps.tile([128, 500], FP, tag="pb", name="ps", bufs=3)
                    for f in range(4):
                        nc.tensor.matmul(out=ps[:mw, :],
                                         lhsT=tT[f][:, mo:mo + mw],
                                         rhs=wo_t[f][:, :500],
                                         start=(f == 0), stop=(f == 3))
                    dv = tsc.tile([128, 500], FP, tag="scr", name="dv")
                    nc.vector.tensor_tensor(
                        out=dv[:mw, :], in0=ps[:mw, :],
                        in1=bo_sl[:mw, :], op=ALU.add)
                    nc.sync.dma_start(
                        out=dec_o[mo:mo + mw, nchi * 500:(nchi + 1) * 500],
                        in_=dv[:mw, :])
                    mo += mw

            # ==================================================== P6 read
            rsr_sl = []
            for m, mw in enumerate(MCH):
                t = tsm.tile([128, 1], FP, tag=f"rsr{m}", name="t", bufs=1)
                nc.sync.dma_start(out=t[:mw, :],
                                  in_=rsr[m * 128:m * 128 + mw, :])
                rsr_sl.append(t)
            _ro_tags = {(0, 0): ("pb", 3), (0, 1): ("pb", 3), (0, 2): ("pb", 3),
                        (1, 0): ("po", 1), (1, 1): ("gp", 2), (1, 2): ("psx", 2)}
            psro = {(br, m): tps.tile([128, D], FP, name=f"ro{br}{m}",
                                      tag=_ro_tags[br, m][0],
                                      bufs=_ro_tags[br, m][1])
                    for br in (0, 1) for m in range(3)}
            for tb in range(24):
                blk, sub = tb // 3, tb % 3
                toff, tsubw = sub * 128, MCH[sub]
                at = tsc.tile([128, R], BF, tag="scb", name="at")
                nc.sync.dma_start(
                    out=at[:tsubw, :],
                    in_=adjT[blk * R + toff:blk * R + toff + tsubw, :])
                for br in (0, 1):
                    et = tsc.tile([128, D], BF, tag="scb2", name="et")
                    nc.sync.dma_start(out=et[:tsubw, :],
                                      in_=emb_ag[blk, br, toff:toff + tsubw, :])
                    mo = 0
                    for m, mw in enumerate(MCH):
                        nc.tensor.matmul(out=psro[br, m][:mw, :],
                                         lhsT=at[:tsubw, mo:mo + mw],
                                         rhs=et[:tsubw, :],
                                         start=(tb == 0), stop=(tb == 23))
                        mo += mw
            for br in (0, 1):
                for m, mw in enumerate(MCH):
                    cta = tsc.tile([128, D], FP, tag="scr", name="cta")
                    nc.vector.tensor_scalar_mul(out=cta[:mw, :],
                                                in0=psro[br, m][:mw, :],
                                                scalar1=rsr_sl[m][:mw, :1])
                    sq = tsc.tile([128, D], FP, tag="scr", name="sq")
                    nc.vector.tensor_tensor(out=sq[:mw, :], in0=cta[:mw, :],
                                            in1=cta[:mw, :], op=ALU.mult)
                    ss = tsm.tile([128, 1], FP, tag="ss", name="ss")
                    nc.vector.reduce_sum(out=ss[:mw, :], in_=sq[:mw, :],
                                         axis=AXX)
                    sr = tsm.tile([128, 1], FP, tag="sr", name="sr")
                    nc.scalar.sqrt(out=sr[:mw, :], in_=ss[:mw, :])
                    smx = tsm.tile([128, 1], FP, tag="smx", name="smx")
                    nc.vector.tensor_scalar_max(out=smx[:mw, :],
                                                in0=sr[:mw, :], scalar1=1e-12)
                    rn = tsm.tile([128, 1], FP, tag="rn", name="rn")
                    nc.vector.reciprocal(out=rn[:mw, :], in_=smx[:mw, :])
                    cn = tsc.tile([128, D], FP, tag="scr", name="cn")
                    nc.vector.tensor_scalar_mul(out=cn[:mw, :],
                                                in0=cta[:mw, :],
                                                scalar1=rn[:mw, :1])
                    nc.scalar.activation(out=g_ro[br][m][:mw, :],
                                         in_=cn[:mw, :], func=AF.Sigmoid)

            # ==================================================== P7 disc
            wd = []
            for e in range(4):
                t = tws.tile([128, D], BF, tag="w", name="wt")
                nc.sync.dma_start(out=t[:], in_=WdT[e * 128:(e + 1) * 128, :])
                wd.append(t)
            for br, out_t in ((0, lg_o), (1, lga_o)):
                transpose_rows(tps, g_ro[br], tT, pbufs=3)
                mo = 0
                for m, mw in enumerate(MCH):
                    ps = tps.tile([128, D], FP, tag="pb", name="ps", bufs=3)
                    for e in range(4):
                        nc.tensor.matmul(out=ps[:mw, :],
                                         lhsT=tT[e][:, mo:mo + mw],
                                         rhs=wd[e][:, :D],
                                         start=(e == 0), stop=(e == 3))
                    t1 = tsc.tile([128, D], FP, tag="scr", name="t1")
                    nc.vector.tensor_copy(out=t1[:mw, :], in_=ps[:mw, :])
                    lgt = tsm.tile([128, 2], FP, tag="lgt", name="lgt")
                    for col, ebr in ((0, br), (1, 1 - br)):
                        pr = tsc.tile([128, D], FP, tag="scr", name="pr")
                        nc.vector.tensor_tensor(out=pr[:mw, :],
                                                in0=emb_bf[ebr][m][:mw, :],
                                                in1=t1[:mw, :], op=ALU.mult)
                        s1 = tsm.tile([128, 1], FP, tag="s1", name="s1")
                        nc.vector.reduce_sum(out=s1[:mw, :], in_=pr[:mw, :],
                                             axis=AXX)
                        nc.vector.tensor_scalar_add(
                            out=lgt[:mw, col:col + 1], in0=s1[:mw, :],
                            scalar1=sca_t[:mw, 2:3])
                    nc.sync.dma_start(out=out_t[mo:mo + mw, :],
                                      in_=lgt[:mw, :])
                    mo += mw
    nc.compile()
    return nc


# ------------------------------------------------------------------ host API
def _pack_rows(inputs):
    f32 = np.float32
    z = []
    z.append(np.asarray(inputs["bproj"], f32).reshape(-1))
    z.append(np.asarray(inputs["att"], f32).reshape(-1))
    z.append(np.asarray(inputs["gat_b"], f32).reshape(-1))
    bqkv = np.asarray(inputs["bqkv"], f32)
    z.append(bqkv[0, 2 * D:3 * D]); z.append(bqkv[1, 2 * D:3 * D])
    bo = np.asarray(inputs["bo"], f32)
    z.append(bo[0]); z.append(bo[1])
    bff2 = np.asarray(inputs["bff2"], f32)
    z.append(bff2[0]); z.append(bff2[1])
    for l in range(2):
        z.append(np.asarray(inputs["ln1_g"], f32)[l])
        z.append(np.asarray(inputs["ln1_b"], f32)[l])
        z.append(np.asarray(inputs["ln2_g"], f32)[l])
        z.append(np.asarray(inputs["ln2_b"], f32)[l])
    z.append(np.asarray(inputs["bout"], f32).reshape(-1))
    return np.concatenate(z)[None, :]


def _make_in_maps(inputs):
    f32 = np.float32
    prep, nbins = _prep_edges(inputs["edge_index"])

    feat = np.asarray(inputs["feat"], f32)
    feat_a = np.asarray(inputs["feat_a"], f32)
    adj = np.asarray(inputs["adj_new"], f32)
    rows = _pack_rows(inputs).astype(BF_NP)
    a = 1.0 / (1.0 + np.exp(-float(np.asarray(inputs["alpha_param"]).reshape(-1)[0])))
    sca = np.array([[a, 1.0 - a, float(np.asarray(inputs["bdisc"])), 0.0]], f32)

    bf = BF_NP
    Wl = np.asarray(inputs["Wl"], f32).astype(bf)
    Wr = np.asarray(inputs["Wr"], f32).astype(bf)
    Wp = np.asarray(inputs["Wproj"], f32).astype(bf)
    Wqkv = np.asarray(inputs["Wqkv"], f32)
    WqkvT = np.ascontiguousarray(np.transpose(Wqkv, (0, 2, 1))).astype(bf)
    bqkvT = np.ascontiguousarray(np.asarray(inputs["bqkv"], f32)[:, :, None])
    Wo = np.asarray(inputs["Wo"], f32)
    WoT = np.ascontiguousarray(np.transpose(Wo, (0, 2, 1))).astype(bf)
    Wff1 = np.asarray(inputs["Wff1"], f32).astype(bf)
    bff1T = np.ascontiguousarray(np.asarray(inputs["bff1"], f32)[:, :, None])
    Wff2 = np.asarray(inputs["Wff2"], f32).astype(bf)
    Wout = np.asarray(inputs["Wout"], f32).astype(bf)
    WdiscT = np.ascontiguousarray(np.asarray(inputs["Wdisc"], f32).T).astype(bf)

    in_maps = []
    for k in range(NCORES):
        lo = k * R
        p = prep[k]
        sg = p["srcg"][:, :, 0].astype(np.int64)
        blk, subr = sg // R, sg % R
        srcA = np.ascontiguousarray((blk * R + subr).astype(np.int32).T)
        srcB = srcA
        rs = adj[lo:lo + R].sum(1, keepdims=True)
        in_maps.append(dict(
            xTa=np.ascontiguousarray(feat[lo:lo + R].T).astype(bf),
            xTb=np.ascontiguousarray(feat_a[lo:lo + R].T).astype(bf),
            adjT=np.ascontiguousarray(adj[lo:lo + R].T).astype(bf),
            rsrecip=(1.0 / rs).astype(f32),
            Wl=Wl, Wr=Wr, Wproj=Wp, WqkvT=WqkvT, bqkvT=bqkvT, WoT=WoT,
            Wff1=Wff1, bff1T=bff1T, Wff2=Wff2, Wout=Wout, WdiscT=WdiscT,
            rows=rows, sca=sca,
            Qm=p["Q"], Epm=p["Ep"], Ptm=p["Pt"], srcA=srcA, srcB=srcB,
            logm=p["logm"],
        ))
    return in_maps, nbins


def kernel(**inputs):
    in_maps, nbins = _make_in_maps(inputs)
    if nbins not in _CACHE:
        _CACHE[nbins] = _build(nbins)
    nc = _CACHE[nbins]
    trace = bool(os.environ.get("KERNEL_TRACE"))
    res = run_bass_kernel_spmd(nc, in_maps, core_ids=list(range(NCORES)),
                               trace=trace)
    global LAST_RESULT
    LAST_RESULT = res
    outs = res.results
    emb = np.concatenate([outs[k]["emb_o"] for k in range(NCORES)], 0)
    dec = np.concatenate([outs[k]["dec_o"] for k in range(NCORES)], 0)
    lg = np.concatenate([outs[k]["lg_o"] for k in range(NCORES)], 0)
    lga = np.concatenate([outs[k]["lga_o"] for k in range(NCORES)], 0)
    return emb, dec, lg, lga


def bench(iters=8, **inputs):
    """Device-resident repeated execution timing (ns, min over iters)."""
    import time as _time
    import jax
    from jax.sharding import Mesh, PartitionSpec, NamedSharding
    from jax.experimental.shard_map import shard_map
    from concourse import bass2jax

    in_maps, nbins = _make_in_maps(inputs)
    if nbins not in _CACHE:
        _CACHE[nbins] = _build(nbins)
    nc = _CACHE[nbins]
    bass2jax.install_neuronx_cc_hook()
    pname = nc.partition_id_tensor.name if nc.partition_id_tensor else None
    in_names, out_names, out_avals, zeros = [], [], [], []
    for alloc in nc.m.functions[0].allocations:
        if not isinstance(alloc, mybir.MemoryLocationSet):
            continue
        name = alloc.memorylocations[0].name
        if alloc.kind == "ExternalInput":
            if name != pname:
                in_names.append(name)
        elif alloc.kind == "ExternalOutput":
            out_names.append(name)
            shape = tuple(alloc.tensor_shape)
            dtype = mybir.dt.np(alloc.dtype)
            out_avals.append(jax.core.ShapedArray(shape, dtype))
            zeros.append(np.zeros(shape, dtype))
    n_params = len(in_names)
    all_names = in_names + out_names + ([pname] if pname else [])

    def _body(*args):
        ops = list(args)
        if pname:
            ops.append(bass2jax.partition_id_tensor())
        return tuple(bass2jax._bass_exec_p.bind(
            *ops, out_avals=tuple(out_avals), in_names=tuple(all_names),
            out_names=tuple(out_names), lowering_input_output_aliases=(),
            sim_require_finite=True, sim_require_nnan=True, nc=nc))

    devices = jax.devices()[:NCORES]
    mesh = Mesh(np.asarray(devices), ("core",))
    nio = n_params + len(out_avals)
    fn = jax.jit(shard_map(_body, mesh=mesh,
                           in_specs=(PartitionSpec("core"),) * nio,
                           out_specs=(PartitionSpec("core"),) * len(out_avals),
                           check_rep=False), keep_unused=True)
    per_core = [[np.asarray(m[nm]) for nm in in_names] for m in in_maps]
    cat = [np.concatenate([per_core[c][i] for c in range(NCORES)], axis=0)
           for i in range(n_params)]
    catz = [np.zeros((NCORES * z.shape[0], *z.shape[1:]), z.dtype)
            for z in zeros]
    sh = NamedSharding(mesh, PartitionSpec("core"))
    din_ = [jax.device_put(x, sh) for x in cat]
    dz = [jax.device_put(x, sh) for x in catz]
    r = fn(*din_, *dz)
    jax.block_until_ready(r)
    times = []
    for _ in range(iters):
        t0 = _time.perf_counter()
        r = fn(*din_, *dz)
        jax.block_until_ready(r)
        times.append((_time.perf_counter() - t0) * 1e9)
    outs = [np.asarray(r[i]).reshape(NCORES, *out_avals[i].shape)
            for i in range(len(out_names))]
    res = {nm: outs[i].reshape(-1, *out_avals[i].shape[1:])
           for i, nm in enumerate(out_names)}
    return res, min(times)
